# revision 1
# baseline (speedup 1.0000x reference)
"""MoE feed-forward (top-1 routing) Trainium2 kernel.

Strategy (v3: hidden-dim sharding)
----------------------------------
Host: gate logits + argmax replicated bit-exactly with jax on CPU (the
  reference's own op sequence), so routing always matches the oracle.
  Tokens are sorted by expert; every core sees ALL tokens.
Device (single pass, 8 cores, F-parallel): core c owns a 512-wide slice
  of the hidden dimension F for ALL experts (weights: 8.4 MB fp16 per
  core -- each expert's weights are read exactly once chip-wide).
  Per core: h[f_slice] = gelu(W1[:, f_slice]^T X^T + b1[f_slice]) for
  every token (using that token's expert weights), then the partial
  Y^T = W2[f_slice, :]^T h[f_slice] is DMA'd out in fp16.
  This is perfectly load-balanced regardless of gate skew: no padding,
  every core streams exactly T columns through the PE per layer.
Host: sum the 8 fp16 partials in fp32, add b2, scatter back to [B,L,D].
"""

import sys

if "/opt/trn_rl_repo" not in sys.path:
    sys.path.insert(0, "/opt/trn_rl_repo")

import numpy as np

import concourse.bacc as bacc
import concourse.mybir as mybir
import concourse.tile as tile

D, F, E = 1024, 4096, 4
B, L = 4, 2048
T = B * L
NC = 8
P = 128
KD = D // P          # 8  k-tiles over D
FSL = F // NC        # 512 f-slice per core
NFL = FSL // P       # 4  f-tiles per core
GROUP = 1024         # token-column group (x-load / L2-psum unit)
NG = T // GROUP      # 8 groups

TRACE = False
TRACE_CORES = None
LAST_EXEC_NS = []
LAST_TRACES = []
LAST_RESULTS = []

_cache = {}


def _run(nc, in_maps):
    import os
    import time

    from concourse import bass_utils

    trace = TRACE
    if trace:
        bass_utils.upload_artifacts = lambda d: "local://" + d

    def _go(tr):
        return bass_utils.run_bass_kernel_spmd(
            nc, in_maps, core_ids=list(range(NC)), trace=tr,
            trace_cores=TRACE_CORES,
        )

    def _go_untraced():
        # tracing infra broken (hook import failed): force-disable it for
        # the retry — run_bass_kernel_spmd also ORs in the BASS_TRACE env
        prev = os.environ.get("BASS_NEVER_TRACE")
        os.environ["BASS_NEVER_TRACE"] = "1"
        try:
            return _go(False)
        finally:
            if prev is None:
                os.environ.pop("BASS_NEVER_TRACE", None)
            else:
                os.environ["BASS_NEVER_TRACE"] = prev

    res = None
    for attempt in range(3):
        try:
            res = _go(trace)
            break
        except ModuleNotFoundError:
            trace = False
            res = _go_untraced()
            break
        except Exception as ex:
            # the device occasionally comes up wedged
            # (NRT_EXEC_UNIT_UNRECOVERABLE); retry, resetting the jax
            # backend so the retry gets a fresh PJRT client
            msg = str(ex)
            retriable = "UNRECOVERABLE" in msg or "UNAVAILABLE" in msg
            if attempt == 2 or not retriable:
                raise
            try:
                import jax
                import jax.extend.backend as _jeb

                jax.clear_caches()
                _jeb.clear_backends()
            except Exception:
                pass
            time.sleep(2.0)
    if trace:
        LAST_EXEC_NS.append(res.exec_time_ns)
        LAST_TRACES.append(
            res.instructions_and_trace[1] if res.instructions_and_trace else None
        )
        LAST_RESULTS.append(res)
    return res


def _pieces(counts):
    """Cut the expert-sorted token axis into (expert, col0, len) pieces that
    respect both the 512 grid (PSUM bank) and expert boundaries."""
    bounds = np.cumsum(counts)
    pieces = []
    pos = 0
    for e in range(E):
        end = int(bounds[e])
        while pos < end:
            nxt = min(end, (pos // 512 + 1) * 512)
            pieces.append((e, pos, nxt - pos))
            pos = nxt
    return pieces


def _build_ffn(counts):
    """counts: per-expert token counts (tuple of 4 ints summing to T)."""
    key = ("ffn3", counts)
    if key in _cache:
        return _cache[key]
    f32 = mybir.dt.float32
    f16 = mybir.dt.float16
    pieces = _pieces(counts)
    nc = bacc.Bacc("TRN2", target_bir_lowering=False, debug=False, num_devices=NC)
    xt = nc.dram_tensor("xt", (D, T), f16, kind="ExternalInput")
    # per-core f-slice weights, pre-arranged so the partition dim is first
    w1 = nc.dram_tensor("w1", (P, E, NFL, KD, P), f16, kind="ExternalInput")
    b1 = nc.dram_tensor("b1", (P, E, NFL), f32, kind="ExternalInput")
    w2 = nc.dram_tensor("w2", (P, E, KD, NFL, P), f16, kind="ExternalInput")
    yt = nc.dram_tensor("yt", (D, T), f16, kind="ExternalOutput")

    with tile.TileContext(nc) as tc:
        with (
            tc.tile_pool(name="xs", bufs=3) as xpool,
            tc.tile_pool(name="hs", bufs=1) as hpool,
            tc.tile_pool(name="wp", bufs=1) as wpool,
            tc.tile_pool(name="yp", bufs=6) as ypool,
        ):
            # One PSUM pool for everything: p1_0..3 (L1) + p2_0..3 (L2) = 8
            # banks, so no pool-transition barrier separates the layers.
            # Warm the PE clock (HAM un-throttles 1.2->2.4 GHz after ~3.4us
            # of sustained activity) with dummy matmuls into a p2 bank while
            # the first DMAs are still in flight.
            ps_cm = tc.tile_pool(name="ps", bufs=1, space="PSUM")
            psum = ps_cm.__enter__()
            wsrc = wpool.tile([P, 512], f16, name="wsrc")
            nc.vector.memset(wsrc[:], 0.0)
            wdst = psum.tile([P, 512], f32, name="p2_0")
            warm_on = True
            # all 9 bridge dummies BEFORE the first real chain: the PE queue
            # is in-order, so dummies emitted after the data-gated first
            # matmul could not cover late DMA arrival on hardware
            for _ in range(9):
                nc.tensor.matmul(
                    wdst[:], wsrc[:, :P], wsrc[:], start=True, stop=True
                )
            # pre-load the Gelu table on the Act engine while DMAs stream
            gwarm = wpool.tile([P, 8], f16, name="gwarm")
            nc.scalar.activation(
                gwarm[:], wsrc[:, :8], mybir.ActivationFunctionType.Gelu
            )

            # weights: one tile per expert so the first expert lands first
            e_first = pieces[0][0]
            e_order = [e_first] + [e for e in range(E) if e != e_first]
            w1sb = wpool.tile([P, E, NFL, KD, P], f16, name="w1sb")
            b1sb = wpool.tile([P, E, NFL], f32, name="b1sb")
            w2sb = wpool.tile([P, E, KD, NFL, P], f16, name="w2sb")
            # tiny dedicated tile for the very first stationary set (e0,fl0)
            # so the first real chain isn't gated on the 2.1MB expert DMA
            w1f0 = wpool.tile([P, KD, P], f16, name="w1f0")
            # first x group (k=0 slice first: the first chain needs only it)
            xgs = []
            xg0 = [xpool.tile([P, GROUP], f16, name=f"x_{k}") for k in range(KD)]
            # first k-slice in two halves so the first chain starts sooner
            nc.sync.dma_start(xg0[0][:, 0:512], xt.ap()[0:P, 0:512])
            nc.sync.dma_start(xg0[0][:, 512:GROUP], xt.ap()[0:P, 512:GROUP])
            nc.gpsimd.dma_start(w1f0[:], w1.ap()[:, e_first, 0])
            nc.gpsimd.dma_start(b1sb[:], b1.ap()[:])
            nc.gpsimd.dma_start(w1sb[:, e_first], w1.ap()[:, e_first])
            for k in range(1, KD):
                nc.sync.dma_start(
                    xg0[k][:], xt.ap()[k * P:(k + 1) * P, 0:GROUP]
                )
            for e in e_order[1:]:
                nc.gpsimd.dma_start(w1sb[:, e], w1.ap()[:, e])
            # w2 also on gpsimd: its big strided DMAs cost ~3us of issue
            # time each and would starve x (sync) or the acts (scalar)
            for e in e_order:
                nc.gpsimd.dma_start(w2sb[:, e], w2.ap()[:, e])
            xgs.append(xg0)

            # h[fl] spans all T columns, fp16
            hts = [hpool.tile([P, T], f16, name=f"h{fl}") for fl in range(NFL)]

            # -- layer 1 --
            by_group = [[] for _ in range(NG)]
            for pc in pieces:
                by_group[pc[1] // GROUP].append(pc)
            first = True
            for g in range(NG):
                if g > 0:
                    xg = [
                        xpool.tile([P, GROUP], f16, name=f"x_{k}")
                        for k in range(KD)
                    ]
                    for k in range(KD):
                        nc.sync.dma_start(
                            xg[k][:],
                            xt.ap()[k * P:(k + 1) * P,
                                    g * GROUP:(g + 1) * GROUP],
                        )
                    xgs.append(xg)
                xg = xgs[g]
                for (e, c0, ln) in by_group[g]:
                    lo = c0 - g * GROUP
                    pts = [
                        psum.tile([P, 512], f32, name=f"p1_{fl}")
                        for fl in range(NFL)
                    ]
                    for fl in range(NFL):
                        w1st = w1f0 if first and fl == 0 else w1sb[:, e, fl]
                        for k in range(KD):
                            nc.tensor.matmul(
                                pts[fl][:, :ln], w1st[:, k],
                                xg[k][:, lo:lo + ln],
                                start=(k == 0), stop=(k == KD - 1),
                            )
                        first = False
                        nc.scalar.activation(
                            hts[fl][:, c0:c0 + ln], pts[fl][:, :ln],
                            mybir.ActivationFunctionType.Gelu,
                            bias=b1sb[:, e, fl:fl + 1], scale=1.0,
                        )
            warm_on = False

            # -- layer 2: partial Y^T[d] = sum_fl W2[fl,d]^T H^T[fl] --
            # one chain per (d, piece); copies alternate DVE/Act and stores
            # alternate sync/scalar so no single engine or queue bottlenecks
            l2ps = list(pieces)
            # split the very last piece so the drain ends on small transfers
            (le, lc0, lln) = l2ps[-1]
            if lln > 192:
                h1 = lln - 128
                l2ps[-1:] = [(le, lc0, h1), (le, lc0 + h1, lln - h1)]
            jj = 0
            for d in range(KD):
                plist = l2ps if d == KD - 1 else pieces
                for (e, c0, ln) in plist:
                    pt = psum.tile([P, 512], f32, name=f"p2_{jj % 4}")
                    for fl in range(NFL):
                        nc.tensor.matmul(
                            pt[:, :ln], w2sb[:, e, d, fl],
                            hts[fl][:, c0:c0 + ln],
                            start=(fl == 0), stop=(fl == NFL - 1),
                        )
                    ys = ypool.tile([P, 512], f16, name="ysb")
                    # copies on DVE/Act (GPSIMD cannot read PSUM); stores
                    # rotate over three queues, with the last two pinned to
                    # the HWDGE queues (SP/Act) so the drain never waits on
                    # the software DGE
                    tail2 = d == KD - 1 and (e, c0, ln) in plist[-2:]
                    sel = (len(plist) - plist.index((e, c0, ln))) % 2 \
                        if tail2 else jj % 3
                    if sel % 2 == 0:
                        nc.vector.tensor_scalar_add(
                            ys[:, :ln], pt[:, :ln], 0.0
                        )
                    else:
                        nc.scalar.copy(ys[:, :ln], pt[:, :ln])
                    [nc.sync, nc.scalar, nc.gpsimd][sel].dma_start(
                        yt.ap()[d * P:(d + 1) * P, c0:c0 + ln],
                        ys[:, :ln],
                    )
                    jj += 1
            ps_cm.__exit__(None, None, None)
    nc.compile()
    _cache[key] = nc
    return nc


def _gate_idx(x, Wg, bg):
    """Replicate the reference gate decision exactly (jax on CPU)."""
    try:
        import jax
        import jax.numpy as jnp

        with jax.default_device(jax.devices("cpu")[0]):
            gl = jnp.einsum(
                "bld,de->ble",
                jnp.asarray(x, dtype=jnp.float32),
                jnp.asarray(Wg, dtype=jnp.float32),
            ) + jnp.asarray(bg, dtype=jnp.float32)
            idx = jnp.argmax(jax.nn.softmax(gl, axis=-1), axis=-1)
        return np.asarray(idx).reshape(T)
    except Exception:
        # fallback: float64 argmax (ties at <1e-7 margin are astronomically
        # unlikely with this data distribution)
        xf = np.asarray(x, dtype=np.float64).reshape(T, D)
        gl = xf @ np.asarray(Wg, dtype=np.float64) + np.asarray(bg, np.float64)
        return np.argmax(gl, axis=1)


def kernel(x, W1, b1, W2, b2, Wg, bg):
    x = np.asarray(x, dtype=np.float32)
    W1 = np.asarray(W1, dtype=np.float32)
    b1 = np.asarray(b1, dtype=np.float32)
    W2 = np.asarray(W2, dtype=np.float32)
    b2 = np.asarray(b2, dtype=np.float32)
    Wg = np.asarray(Wg, dtype=np.float32)
    bg = np.asarray(bg, dtype=np.float32)

    xf = x.reshape(T, D)

    # ---- host routing (exact) ----
    idx = _gate_idx(x, Wg, bg)
    order = np.argsort(idx, kind="stable")
    counts = tuple(int(c) for c in np.bincount(idx, minlength=E))

    XT16 = np.ascontiguousarray(xf[order].T.astype(np.float16))  # [D, T] sorted

    # per-core f-slice weight tensors, partition dim first
    # W1: [E, D, F] -> [P(k in tile), E, NC, NFL, KD, P(f)]
    W1t = W1.reshape(E, KD, P, NC, NFL, P).transpose(2, 0, 3, 4, 1, 5)
    W1t = np.ascontiguousarray(W1t).astype(np.float16)  # [P, E, NC, NFL, KD, P]
    b1t = b1.reshape(E, NC, NFL, P).transpose(3, 0, 1, 2)
    b1t = np.ascontiguousarray(b1t)                      # [P, E, NC, NFL]
    # W2: [E, F, D] -> [P(f in tile), E, NC, KD, NFL, P(d)]
    W2t = W2.reshape(E, NC, NFL, P, KD, P).transpose(3, 0, 1, 4, 2, 5)
    W2t = np.ascontiguousarray(W2t).astype(np.float16)  # [P, E, NC, KD, NFL, P]

    in_maps = []
    for c in range(NC):
        in_maps.append({
            "xt": XT16,
            "w1": np.ascontiguousarray(W1t[:, :, c]),
            "b1": np.ascontiguousarray(b1t[:, :, c]),
            "w2": np.ascontiguousarray(W2t[:, :, c]),
        })

    nc2 = _build_ffn(counts)
    res = _run(nc2, in_maps)

    # ---- host reduction: sum partials (fp32), add b2, unsort ----
    acc = res.results[0]["yt"].astype(np.float32)
    for c in range(1, NC):
        acc += res.results[c]["yt"].astype(np.float32)
    ys = acc.T                               # [T, D] in sorted order
    ys += b2[idx[order]]
    out = np.empty((T, D), dtype=np.float32)
    out[order] = ys
    return out.reshape(B, L, D)



# revision 4
# speedup vs baseline: 1.2006x; 1.2006x over previous
"""MoE feed-forward (top-1 routing) Trainium2 kernel.

Strategy (v3: hidden-dim sharding)
----------------------------------
Host: gate logits + argmax replicated bit-exactly with jax on CPU (the
  reference's own op sequence), so routing always matches the oracle.
  Tokens are sorted by expert; every core sees ALL tokens.
Device (single pass, 8 cores, F-parallel): core c owns a 512-wide slice
  of the hidden dimension F for ALL experts (weights: 8.4 MB fp16 per
  core -- each expert's weights are read exactly once chip-wide).
  Per core: h[f_slice] = gelu(W1[:, f_slice]^T X^T + b1[f_slice]) for
  every token (using that token's expert weights), then the partial
  Y^T = W2[f_slice, :]^T h[f_slice] is DMA'd out in fp16.
  This is perfectly load-balanced regardless of gate skew: no padding,
  every core streams exactly T columns through the PE per layer.
Host: sum the 8 fp16 partials in fp32, add b2, scatter back to [B,L,D].
"""

import sys

if "/opt/trn_rl_repo" not in sys.path:
    sys.path.insert(0, "/opt/trn_rl_repo")

import numpy as np

import concourse.bacc as bacc
import concourse.mybir as mybir
import concourse.tile as tile

D, F, E = 1024, 4096, 4
B, L = 4, 2048
T = B * L
NC = 8
P = 128
KD = D // P          # 8  k-tiles over D
FSL = F // NC        # 512 f-slice per core
NFL = FSL // P       # 4  f-tiles per core
GROUP = 1024         # token-column group (x-load / L2-psum unit)
NG = T // GROUP      # 8 groups

TRACE = False
TRACE_CORES = None
LAST_EXEC_NS = []
LAST_TRACES = []
LAST_RESULTS = []

_cache = {}


def _run(nc, in_maps):
    import os
    import time

    from concourse import bass_utils

    trace = TRACE
    if trace:
        bass_utils.upload_artifacts = lambda d: "local://" + d

    def _go(tr):
        return bass_utils.run_bass_kernel_spmd(
            nc, in_maps, core_ids=list(range(NC)), trace=tr,
            trace_cores=TRACE_CORES,
        )

    def _go_untraced():
        # tracing infra broken (hook import failed): force-disable it for
        # the retry — run_bass_kernel_spmd also ORs in the BASS_TRACE env
        prev = os.environ.get("BASS_NEVER_TRACE")
        os.environ["BASS_NEVER_TRACE"] = "1"
        try:
            return _go(False)
        finally:
            if prev is None:
                os.environ.pop("BASS_NEVER_TRACE", None)
            else:
                os.environ["BASS_NEVER_TRACE"] = prev

    res = None
    for attempt in range(3):
        try:
            res = _go(trace)
            break
        except ModuleNotFoundError:
            trace = False
            res = _go_untraced()
            break
        except Exception as ex:
            # the device occasionally comes up wedged
            # (NRT_EXEC_UNIT_UNRECOVERABLE); retry, resetting the jax
            # backend so the retry gets a fresh PJRT client
            msg = str(ex)
            retriable = "UNRECOVERABLE" in msg or "UNAVAILABLE" in msg
            if attempt == 2 or not retriable:
                raise
            try:
                import jax
                import jax.extend.backend as _jeb

                jax.clear_caches()
                _jeb.clear_backends()
            except Exception:
                pass
            time.sleep(2.0)
    if trace:
        LAST_EXEC_NS.append(res.exec_time_ns)
        LAST_TRACES.append(
            res.instructions_and_trace[1] if res.instructions_and_trace else None
        )
        LAST_RESULTS.append(res)
    return res


def _pieces(counts):
    """Cut the expert-sorted token axis into (expert, col0, len) pieces that
    respect both the 512 grid (PSUM bank) and expert boundaries."""
    bounds = np.cumsum(counts)
    pieces = []
    pos = 0
    for e in range(E):
        end = int(bounds[e])
        while pos < end:
            nxt = min(end, (pos // 512 + 1) * 512)
            pieces.append((e, pos, nxt - pos))
            pos = nxt
    return pieces


def _build_ffn(counts):
    """counts: per-expert token counts (tuple of 4 ints summing to T)."""
    key = ("ffn3", counts)
    if key in _cache:
        return _cache[key]
    f32 = mybir.dt.float32
    f16 = mybir.dt.float16
    pieces = _pieces(counts)
    nc = bacc.Bacc("TRN2", target_bir_lowering=False, debug=False, num_devices=NC)
    xt = nc.dram_tensor("xt", (D, T), f16, kind="ExternalInput")
    # per-core f-slice weights, pre-arranged so the partition dim is first
    w1 = nc.dram_tensor("w1", (P, E, NFL, KD, P), f16, kind="ExternalInput")
    b1 = nc.dram_tensor("b1", (P, E, NFL), f32, kind="ExternalInput")
    w2 = nc.dram_tensor("w2", (P, E, KD, NFL, P), f16, kind="ExternalInput")
    yt = nc.dram_tensor("yt", (D, T), f16, kind="ExternalOutput")

    with tile.TileContext(nc) as tc:
        with (
            tc.tile_pool(name="xs", bufs=3) as xpool,
            tc.tile_pool(name="hs", bufs=1) as hpool,
            tc.tile_pool(name="wp", bufs=1) as wpool,
            tc.tile_pool(name="yp", bufs=6) as ypool,
        ):
            # One PSUM pool for everything: p1_0..3 (L1) + p2_0..3 (L2) = 8
            # banks, so no pool-transition barrier separates the layers.
            # Warm the PE clock (HAM un-throttles after ~3.4us of sustained
            # activity) with dummy matmuls into a p2 bank while the first
            # DMAs are still in flight.
            ps_cm = tc.tile_pool(name="ps", bufs=1, space="PSUM")
            psum = ps_cm.__enter__()
            wsrc = wpool.tile([P, 512], f16, name="wsrc")
            nc.vector.memset(wsrc[:], 0.0)
            wdst = psum.tile([P, 512], f32, name="p2_0")
            # bridge dummies BEFORE the first real chain: the PE queue is
            # in-order, so dummies emitted after the data-gated first matmul
            # could not cover late DMA arrival on hardware. 16 x 512-col
            # dummies ~= 4.2us, matching the ~4-5us arrival of the first
            # 1MB x half-group + first w1 f-slice.
            for _ in range(16):
                nc.tensor.matmul(
                    wdst[:], wsrc[:, :P], wsrc[:], start=True, stop=True
                )
            # pre-load the Gelu table on the Act engine while DMAs stream
            gwarm = wpool.tile([P, 8], f16, name="gwarm")
            nc.scalar.activation(
                gwarm[:], wsrc[:, :8], mybir.ActivationFunctionType.Gelu
            )

            # weights: per-f-slice triggers for the first expert so the
            # first chains are gated on 256KB, not the full 1MB expert
            e_first = pieces[0][0]
            e_order = [e_first] + [e for e in range(E) if e != e_first]
            w1sb = wpool.tile([P, E, NFL, KD, P], f16, name="w1sb")
            b1sb = wpool.tile([P, E, NFL], f32, name="b1sb")
            w2sb = wpool.tile([P, E, KD, NFL, P], f16, name="w2sb")
            nc.gpsimd.dma_start(w1sb[:, e_first, 0], w1.ap()[:, e_first, 0])
            nc.gpsimd.dma_start(b1sb[:], b1.ap()[:])
            for fl in range(1, NFL):
                nc.gpsimd.dma_start(
                    w1sb[:, e_first, fl], w1.ap()[:, e_first, fl]
                )
            for e in e_order[1:]:
                nc.gpsimd.dma_start(w1sb[:, e], w1.ap()[:, e])
            # w2 triggers are NOT issued here: they are emitted on the Act
            # queue after group 1's gelus (below), so the 4.2MB doesn't
            # compete with x/w1 for HBM during the critical first ~30us.

            # x: one [P, KD, GROUP] tile per group, ONE descriptor each
            # (128x8 rows of 2KB) -- collapses sync-engine trigger cost
            # from 64 to 9 issues. Group 0 in two halves so the first
            # 512-col piece is gated on 1MB only.
            xgs = []
            for g in range(NG):
                xg = xpool.tile([P, KD, GROUP], f16, name="xg")
                src = xt.ap().rearrange(
                    "(kd p) t -> p kd t", p=P
                )[:, :, g * GROUP:(g + 1) * GROUP]
                if g == 0:
                    nc.sync.dma_start(xg[:, :, 0:512], src[:, :, 0:512])
                    nc.sync.dma_start(xg[:, :, 512:GROUP], src[:, :, 512:GROUP])
                else:
                    nc.sync.dma_start(xg[:], src)
                xgs.append(xg)

            # h[fl] spans all T columns, fp16
            hts = [hpool.tile([P, T], f16, name=f"h{fl}") for fl in range(NFL)]

            # -- layer 1 --
            by_group = [[] for _ in range(NG)]
            for pc in pieces:
                by_group[pc[1] // GROUP].append(pc)
            for g in range(NG):
                xg = xgs[g]
                for (e, c0, ln) in by_group[g]:
                    lo = c0 - g * GROUP
                    pts = [
                        psum.tile([P, 512], f32, name=f"p1_{fl}")
                        for fl in range(NFL)
                    ]
                    for fl in range(NFL):
                        for k in range(KD):
                            nc.tensor.matmul(
                                pts[fl][:, :ln], w1sb[:, e, fl, k],
                                xg[:, k, lo:lo + ln],
                                start=(k == 0), stop=(k == KD - 1),
                            )
                        nc.scalar.activation(
                            hts[fl][:, c0:c0 + ln], pts[fl][:, :ln],
                            mybir.ActivationFunctionType.Gelu,
                            bias=b1sb[:, e, fl:fl + 1], scale=1.0,
                        )
                if g == 1:
                    # deferred w2 loads: the Act engine reaches this point
                    # after group 1's gelus (~25us in), so the transfers
                    # land ~40-60us -- well before layer 2 needs them, and
                    # off the critical early window. Act queue is a HW DGE,
                    # so these also drain promptly.
                    for e in e_order:
                        nc.scalar.dma_start(w2sb[:, e], w2.ap()[:, e])

            # -- layer 2: partial Y^T[d] = sum_fl W2[fl,d]^T H^T[fl] --
            # one chain per (d, piece), rotating over ALL 8 psum banks.
            # Copies alternate Act (ACTIVATE-COPY ~440ns) / DVE (~830ns).
            # Only gpsimd/SP/Act can trigger DMAs; gpsimd's software DGE has
            # ~7us completion latency (it WAS the whole end-of-kernel drain
            # in v3), so it only gets stores in the first 6 of 8 d-tiles.
            l2ps = list(pieces)
            # split the very last piece so the drain ends on small transfers
            (le, lc0, lln) = l2ps[-1]
            if lln > 192:
                h1 = lln - 128
                l2ps[-1:] = [(le, lc0, h1), (le, lc0 + h1, lln - h1)]
            banks = ["p2_0", "p2_1", "p2_2", "p2_3",
                     "p1_0", "p1_1", "p1_2", "p1_3"]
            jj = 0
            for d in range(KD):
                plist = l2ps if d == KD - 1 else pieces
                for ip, (e, c0, ln) in enumerate(plist):
                    pt = psum.tile([P, 512], f32, name=banks[jj % 8])
                    for fl in range(NFL):
                        nc.tensor.matmul(
                            pt[:, :ln], w2sb[:, e, d, fl],
                            hts[fl][:, c0:c0 + ln],
                            start=(fl == 0), stop=(fl == NFL - 1),
                        )
                    ys = ypool.tile([P, 512], f16, name="ysb")
                    tail2 = d == KD - 1 and ip >= len(plist) - 2
                    if tail2:
                        # last two stores pinned to the HW DGE queues
                        cp = (len(plist) - ip) % 2
                        st = [nc.sync, nc.scalar][(len(plist) - ip) % 2]
                    elif d < KD - 2:
                        cp = jj % 2
                        st = [nc.sync, nc.scalar, nc.gpsimd][jj % 3]
                    else:
                        cp = jj % 2
                        st = [nc.sync, nc.scalar][jj % 2]
                    if cp == 0:
                        nc.scalar.copy(ys[:, :ln], pt[:, :ln])
                    else:
                        nc.vector.tensor_scalar_add(
                            ys[:, :ln], pt[:, :ln], 0.0
                        )
                    st.dma_start(
                        yt.ap()[d * P:(d + 1) * P, c0:c0 + ln],
                        ys[:, :ln],
                    )
                    jj += 1
            ps_cm.__exit__(None, None, None)
    nc.compile()
    _cache[key] = nc
    return nc


def _gate_idx(x, Wg, bg):
    """Replicate the reference gate decision exactly (jax on CPU)."""
    try:
        import jax
        import jax.numpy as jnp

        with jax.default_device(jax.devices("cpu")[0]):
            gl = jnp.einsum(
                "bld,de->ble",
                jnp.asarray(x, dtype=jnp.float32),
                jnp.asarray(Wg, dtype=jnp.float32),
            ) + jnp.asarray(bg, dtype=jnp.float32)
            idx = jnp.argmax(jax.nn.softmax(gl, axis=-1), axis=-1)
        return np.asarray(idx).reshape(T)
    except Exception:
        # fallback: float64 argmax (ties at <1e-7 margin are astronomically
        # unlikely with this data distribution)
        xf = np.asarray(x, dtype=np.float64).reshape(T, D)
        gl = xf @ np.asarray(Wg, dtype=np.float64) + np.asarray(bg, np.float64)
        return np.argmax(gl, axis=1)


def kernel(x, W1, b1, W2, b2, Wg, bg):
    x = np.asarray(x, dtype=np.float32)
    W1 = np.asarray(W1, dtype=np.float32)
    b1 = np.asarray(b1, dtype=np.float32)
    W2 = np.asarray(W2, dtype=np.float32)
    b2 = np.asarray(b2, dtype=np.float32)
    Wg = np.asarray(Wg, dtype=np.float32)
    bg = np.asarray(bg, dtype=np.float32)

    xf = x.reshape(T, D)

    # ---- host routing (exact) ----
    idx = _gate_idx(x, Wg, bg)
    order = np.argsort(idx, kind="stable")
    counts = tuple(int(c) for c in np.bincount(idx, minlength=E))

    XT16 = np.ascontiguousarray(xf[order].T.astype(np.float16))  # [D, T] sorted

    # per-core f-slice weight tensors, partition dim first
    # W1: [E, D, F] -> [P(k in tile), E, NC, NFL, KD, P(f)]
    W1t = W1.reshape(E, KD, P, NC, NFL, P).transpose(2, 0, 3, 4, 1, 5)
    W1t = np.ascontiguousarray(W1t).astype(np.float16)  # [P, E, NC, NFL, KD, P]
    b1t = b1.reshape(E, NC, NFL, P).transpose(3, 0, 1, 2)
    b1t = np.ascontiguousarray(b1t)                      # [P, E, NC, NFL]
    # W2: [E, F, D] -> [P(f in tile), E, NC, KD, NFL, P(d)]
    W2t = W2.reshape(E, NC, NFL, P, KD, P).transpose(3, 0, 1, 4, 2, 5)
    W2t = np.ascontiguousarray(W2t).astype(np.float16)  # [P, E, NC, KD, NFL, P]

    in_maps = []
    for c in range(NC):
        in_maps.append({
            "xt": XT16,
            "w1": np.ascontiguousarray(W1t[:, :, c]),
            "b1": np.ascontiguousarray(b1t[:, :, c]),
            "w2": np.ascontiguousarray(W2t[:, :, c]),
        })

    nc2 = _build_ffn(counts)
    res = _run(nc2, in_maps)

    # ---- host reduction: sum partials (fp32), add b2, unsort ----
    acc = res.results[0]["yt"].astype(np.float32)
    for c in range(1, NC):
        acc += res.results[c]["yt"].astype(np.float32)
    ys = acc.T                               # [T, D] in sorted order
    ys += b2[idx[order]]
    out = np.empty((T, D), dtype=np.float32)
    out[order] = ys
    return out.reshape(B, L, D)



# revision 10
# speedup vs baseline: 1.2392x; 1.0321x over previous
"""MoE feed-forward (top-1 routing) Trainium2 kernel.

Strategy (v3: hidden-dim sharding)
----------------------------------
Host: gate logits + argmax replicated bit-exactly with jax on CPU (the
  reference's own op sequence), so routing always matches the oracle.
  Tokens are sorted by expert; every core sees ALL tokens.
Device (single pass, 8 cores, F-parallel): core c owns a 512-wide slice
  of the hidden dimension F for ALL experts (weights: 8.4 MB fp16 per
  core -- each expert's weights are read exactly once chip-wide).
  Per core: h[f_slice] = gelu(W1[:, f_slice]^T X^T + b1[f_slice]) for
  every token (using that token's expert weights), then the partial
  Y^T = W2[f_slice, :]^T h[f_slice] is DMA'd out in fp16.
  This is perfectly load-balanced regardless of gate skew: no padding,
  every core streams exactly T columns through the PE per layer.
Host: sum the 8 fp16 partials in fp32, add b2, scatter back to [B,L,D].
"""

import sys

if "/opt/trn_rl_repo" not in sys.path:
    sys.path.insert(0, "/opt/trn_rl_repo")

import numpy as np

import concourse.bacc as bacc
import concourse.mybir as mybir
import concourse.tile as tile

D, F, E = 1024, 4096, 4
B, L = 4, 2048
T = B * L
NC = 8
P = 128
KD = D // P          # 8  k-tiles over D
FSL = F // NC        # 512 f-slice per core
NFL = FSL // P       # 4  f-tiles per core
GROUP = 1024         # token-column group (x-load / L2-psum unit)
NG = T // GROUP      # 8 groups

TRACE = False
TRACE_CORES = None
LAST_EXEC_NS = []
LAST_TRACES = []
LAST_RESULTS = []

_cache = {}


def _run(nc, in_maps):
    import os
    import time

    from concourse import bass_utils

    trace = TRACE
    if trace:
        bass_utils.upload_artifacts = lambda d: "local://" + d

    def _go(tr):
        return bass_utils.run_bass_kernel_spmd(
            nc, in_maps, core_ids=list(range(NC)), trace=tr,
            trace_cores=TRACE_CORES,
        )

    def _go_untraced():
        # tracing infra broken (hook import failed): force-disable it for
        # the retry — run_bass_kernel_spmd also ORs in the BASS_TRACE env
        prev = os.environ.get("BASS_NEVER_TRACE")
        os.environ["BASS_NEVER_TRACE"] = "1"
        try:
            return _go(False)
        finally:
            if prev is None:
                os.environ.pop("BASS_NEVER_TRACE", None)
            else:
                os.environ["BASS_NEVER_TRACE"] = prev

    res = None
    for attempt in range(3):
        try:
            res = _go(trace)
            break
        except ModuleNotFoundError:
            trace = False
            res = _go_untraced()
            break
        except Exception as ex:
            # the device occasionally comes up wedged
            # (NRT_EXEC_UNIT_UNRECOVERABLE); retry, resetting the jax
            # backend so the retry gets a fresh PJRT client
            msg = str(ex)
            retriable = "UNRECOVERABLE" in msg or "UNAVAILABLE" in msg
            if attempt == 2 or not retriable:
                raise
            try:
                import jax
                import jax.extend.backend as _jeb

                jax.clear_caches()
                _jeb.clear_backends()
            except Exception:
                pass
            time.sleep(2.0)
    if trace:
        LAST_EXEC_NS.append(res.exec_time_ns)
        LAST_TRACES.append(
            res.instructions_and_trace[1] if res.instructions_and_trace else None
        )
        LAST_RESULTS.append(res)
    return res


def _pieces(counts):
    """Cut the expert-sorted token axis into (expert, col0, len) pieces that
    respect both the 512 grid (PSUM bank) and expert boundaries."""
    bounds = np.cumsum(counts)
    pieces = []
    pos = 0
    for e in range(E):
        end = int(bounds[e])
        while pos < end:
            nxt = min(end, (pos // 512 + 1) * 512)
            pieces.append((e, pos, nxt - pos))
            pos = nxt
    return pieces


def _build_ffn(counts):
    """counts: per-expert token counts (tuple of 4 ints summing to T)."""
    key = ("ffn3", counts)
    if key in _cache:
        return _cache[key]
    f32 = mybir.dt.float32
    f16 = mybir.dt.float16
    pieces = _pieces(counts)
    nc = bacc.Bacc("TRN2", target_bir_lowering=False, debug=False, num_devices=NC)
    xt = nc.dram_tensor("xt", (D, T), f16, kind="ExternalInput")
    # per-core f-slice weights, pre-arranged so the partition dim is first
    w1 = nc.dram_tensor("w1", (P, E, NFL, KD, P), f16, kind="ExternalInput")
    b1 = nc.dram_tensor("b1", (P, E, NFL), f32, kind="ExternalInput")
    w2 = nc.dram_tensor("w2", (P, E, KD, NFL, P), f16, kind="ExternalInput")
    yt = nc.dram_tensor("yt", (D, T), f16, kind="ExternalOutput")

    with tile.TileContext(nc) as tc:
        with (
            tc.tile_pool(name="xs", bufs=3) as xpool,
            tc.tile_pool(name="hs", bufs=1) as hpool,
            tc.tile_pool(name="wp", bufs=1) as wpool,
            tc.tile_pool(name="yp", bufs=6) as ypool,
        ):
            # One PSUM pool for everything: p1_0..3 (L1) + p2_0..3 (L2) = 8
            # banks, so no pool-transition barrier separates the layers.
            # Warm the PE clock (HAM un-throttles after ~3.4us of sustained
            # activity) with dummy matmuls into a p2 bank while the first
            # DMAs are still in flight.
            ps_cm = tc.tile_pool(name="ps", bufs=1, space="PSUM")
            psum = ps_cm.__enter__()
            # warmup operand memset runs on GpSimd (idle at t=0, no slow
            # TENSOR_LOAD init like DVE), so the dummies are gated only on
            # the Tensor engine's own ~1.2us preamble load.
            wsrc = wpool.tile([P, 512], f16, name="wsrc")
            nc.gpsimd.memset(wsrc[:], 0.0)
            wdst = psum.tile([P, 512], f32, name="p2_0")
            # bridge dummies BEFORE the first real chain: the PE queue is
            # in-order, so dummies emitted after the data-gated first matmul
            # could not cover late DMA arrival on hardware. 16 x 512-col
            # dummies ~= 3.5us, matching the ~4us arrival of the first
            # 1MB x half-group + first w1 f-slice.
            for _ in range(16):
                nc.tensor.matmul(
                    wdst[:], wsrc[:, :P], wsrc[:], start=True, stop=True
                )
            # pre-load the Gelu table on the Act engine while DMAs stream
            gwarm = wpool.tile([P, 8], f16, name="gwarm")
            nc.scalar.activation(
                gwarm[:], wsrc[:, :8], mybir.ActivationFunctionType.Gelu
            )

            # weights: per-f-slice triggers for the first expert so the
            # first chains are gated on 256KB, not the full 1MB expert
            e_first = pieces[0][0]
            e_order = [e_first] + [e for e in range(E) if e != e_first]
            w1sb = wpool.tile([P, E, NFL, KD, P], f16, name="w1sb")
            b1sb = wpool.tile([P, E, NFL], f32, name="b1sb")
            w2sb = wpool.tile([P, E, KD, NFL, P], f16, name="w2sb")
            nc.gpsimd.dma_start(w1sb[:, e_first, 0], w1.ap()[:, e_first, 0])
            nc.gpsimd.dma_start(b1sb[:], b1.ap()[:])
            for fl in range(1, NFL):
                nc.gpsimd.dma_start(
                    w1sb[:, e_first, fl], w1.ap()[:, e_first, fl]
                )
            # w1 for the other experts and all of w2 are NOT issued here:
            # they are emitted on the Act queue after group 0 / group 1's
            # gelus (below), so only ~1MB of weights competes with x for
            # HBM during the critical first ~20us.

            # x: one [P, KD, GROUP] tile per group, loaded as two 512-col
            # half-descriptors (128x8 rows of 1KB each). Region-level deps
            # let each 512-col piece start as soon as its own half landed,
            # halving the starvation quantum vs one 2MB descriptor.
            xgs = []
            for g in range(NG):
                xg = xpool.tile([P, KD, GROUP], f16, name="xg")
                src = xt.ap().rearrange(
                    "(kd p) t -> p kd t", p=P
                )[:, :, g * GROUP:(g + 1) * GROUP]
                nc.sync.dma_start(xg[:, :, 0:512], src[:, :, 0:512])
                nc.sync.dma_start(xg[:, :, 512:GROUP], src[:, :, 512:GROUP])
                xgs.append(xg)

            # h[fl] spans all T columns, fp16
            hts = [hpool.tile([P, T], f16, name=f"h{fl}") for fl in range(NFL)]

            # -- layer 1 --
            by_group = [[] for _ in range(NG)]
            for pc in pieces:
                by_group[pc[1] // GROUP].append(pc)
            for g in range(NG):
                xg = xgs[g]
                for (e, c0, ln) in by_group[g]:
                    lo = c0 - g * GROUP
                    pts = [
                        psum.tile([P, 512], f32, name=f"p1_{fl}")
                        for fl in range(NFL)
                    ]
                    for fl in range(NFL):
                        for k in range(KD):
                            nc.tensor.matmul(
                                pts[fl][:, :ln], w1sb[:, e, fl, k],
                                xg[:, k, lo:lo + ln],
                                start=(k == 0), stop=(k == KD - 1),
                            )
                        nc.scalar.activation(
                            hts[fl][:, c0:c0 + ln], pts[fl][:, :ln],
                            mybir.ActivationFunctionType.Gelu,
                            bias=b1sb[:, e, fl:fl + 1], scale=1.0,
                        )
                if g == 0:
                    # deferred w1 loads for the remaining experts: the Act
                    # engine reaches this point after group 0's gelus
                    # (~15us in); expert e1's first pieces start ~35us.
                    for e in e_order[1:]:
                        nc.scalar.dma_start(w1sb[:, e], w1.ap()[:, e])
                if g == 1:
                    # deferred w2 loads: issued after group 1's gelus
                    # (~25us in), landing well before layer 2 (~130us) and
                    # off the critical early window. Act queue is a HW DGE,
                    # so these also drain promptly.
                    for e in e_order:
                        nc.scalar.dma_start(w2sb[:, e], w2.ap()[:, e])

            # -- layer 2: partial Y^T[d] = sum_fl W2[fl,d]^T H^T[fl] --
            # one chain per (d, piece), rotating over ALL 8 psum banks.
            # Copies alternate Act (ACTIVATE-COPY ~440ns) / DVE (~830ns).
            # Only gpsimd/SP/Act can trigger DMAs; gpsimd's software DGE has
            # ~7us completion latency (it WAS the whole end-of-kernel drain
            # in v3), so it only gets stores in the first 6 of 8 d-tiles.
            l2ps = list(pieces)
            # split the very last piece so the drain ends on small transfers
            (le, lc0, lln) = l2ps[-1]
            if lln > 320:
                h1 = lln - 256
                l2ps[-1:] = [(le, lc0, h1), (le, lc0 + h1, 128),
                             (le, lc0 + h1 + 128, 128)]
            elif lln > 192:
                h1 = lln - 128
                l2ps[-1:] = [(le, lc0, h1), (le, lc0 + h1, lln - h1)]
            banks = ["p2_0", "p2_1", "p2_2", "p2_3",
                     "p1_0", "p1_1", "p1_2", "p1_3"]
            jj = 0
            for d in range(KD):
                plist = l2ps if d == KD - 1 else pieces
                for ip, (e, c0, ln) in enumerate(plist):
                    pt = psum.tile([P, 512], f32, name=banks[jj % 8])
                    for fl in range(NFL):
                        nc.tensor.matmul(
                            pt[:, :ln], w2sb[:, e, d, fl],
                            hts[fl][:, c0:c0 + ln],
                            start=(fl == 0), stop=(fl == NFL - 1),
                        )
                    ys = ypool.tile([P, 512], f16, name="ysb")
                    tail2 = d == KD - 1 and ip >= len(plist) - 3
                    if tail2:
                        # last stores pinned to the HW DGE queues
                        cp = (len(plist) - ip) % 2
                        st = [nc.sync, nc.scalar][(len(plist) - ip) % 2]
                    elif d < KD - 2:
                        cp = jj % 2
                        st = [nc.sync, nc.scalar, nc.gpsimd][jj % 3]
                    else:
                        cp = jj % 2
                        st = [nc.sync, nc.scalar][jj % 2]
                    if cp == 0:
                        nc.scalar.copy(ys[:, :ln], pt[:, :ln])
                    else:
                        nc.vector.tensor_scalar_add(
                            ys[:, :ln], pt[:, :ln], 0.0
                        )
                    st.dma_start(
                        yt.ap()[d * P:(d + 1) * P, c0:c0 + ln],
                        ys[:, :ln],
                    )
                    jj += 1
            ps_cm.__exit__(None, None, None)
    nc.compile()
    _cache[key] = nc
    return nc


def _gate_idx(x, Wg, bg):
    """Replicate the reference gate decision exactly (jax on CPU)."""
    try:
        import jax
        import jax.numpy as jnp

        with jax.default_device(jax.devices("cpu")[0]):
            gl = jnp.einsum(
                "bld,de->ble",
                jnp.asarray(x, dtype=jnp.float32),
                jnp.asarray(Wg, dtype=jnp.float32),
            ) + jnp.asarray(bg, dtype=jnp.float32)
            idx = jnp.argmax(jax.nn.softmax(gl, axis=-1), axis=-1)
        return np.asarray(idx).reshape(T)
    except Exception:
        # fallback: float64 argmax (ties at <1e-7 margin are astronomically
        # unlikely with this data distribution)
        xf = np.asarray(x, dtype=np.float64).reshape(T, D)
        gl = xf @ np.asarray(Wg, dtype=np.float64) + np.asarray(bg, np.float64)
        return np.argmax(gl, axis=1)


def kernel(x, W1, b1, W2, b2, Wg, bg):
    x = np.asarray(x, dtype=np.float32)
    W1 = np.asarray(W1, dtype=np.float32)
    b1 = np.asarray(b1, dtype=np.float32)
    W2 = np.asarray(W2, dtype=np.float32)
    b2 = np.asarray(b2, dtype=np.float32)
    Wg = np.asarray(Wg, dtype=np.float32)
    bg = np.asarray(bg, dtype=np.float32)

    xf = x.reshape(T, D)

    # ---- host routing (exact) ----
    idx = _gate_idx(x, Wg, bg)
    order = np.argsort(idx, kind="stable")
    counts = tuple(int(c) for c in np.bincount(idx, minlength=E))

    XT16 = np.ascontiguousarray(xf[order].T.astype(np.float16))  # [D, T] sorted

    # per-core f-slice weight tensors, partition dim first
    # W1: [E, D, F] -> [P(k in tile), E, NC, NFL, KD, P(f)]
    W1t = W1.reshape(E, KD, P, NC, NFL, P).transpose(2, 0, 3, 4, 1, 5)
    W1t = np.ascontiguousarray(W1t).astype(np.float16)  # [P, E, NC, NFL, KD, P]
    b1t = b1.reshape(E, NC, NFL, P).transpose(3, 0, 1, 2)
    b1t = np.ascontiguousarray(b1t)                      # [P, E, NC, NFL]
    # W2: [E, F, D] -> [P(f in tile), E, NC, KD, NFL, P(d)]
    W2t = W2.reshape(E, NC, NFL, P, KD, P).transpose(3, 0, 1, 4, 2, 5)
    W2t = np.ascontiguousarray(W2t).astype(np.float16)  # [P, E, NC, KD, NFL, P]

    in_maps = []
    for c in range(NC):
        in_maps.append({
            "xt": XT16,
            "w1": np.ascontiguousarray(W1t[:, :, c]),
            "b1": np.ascontiguousarray(b1t[:, :, c]),
            "w2": np.ascontiguousarray(W2t[:, :, c]),
        })

    nc2 = _build_ffn(counts)
    res = _run(nc2, in_maps)

    # ---- host reduction: sum partials (fp32), add b2, unsort ----
    acc = res.results[0]["yt"].astype(np.float32)
    for c in range(1, NC):
        acc += res.results[c]["yt"].astype(np.float32)
    ys = acc.T                               # [T, D] in sorted order
    ys += b2[idx[order]]
    out = np.empty((T, D), dtype=np.float32)
    out[order] = ys
    return out.reshape(B, L, D)



# revision 16
# speedup vs baseline: 1.2671x; 1.0225x over previous
"""MoE feed-forward (top-1 routing) Trainium2 kernel.

Strategy (v3: hidden-dim sharding)
----------------------------------
Host: gate logits + argmax replicated bit-exactly with jax on CPU (the
  reference's own op sequence), so routing always matches the oracle.
  Tokens are sorted by expert; every core sees ALL tokens.
Device (single pass, 8 cores, F-parallel): core c owns a 512-wide slice
  of the hidden dimension F for ALL experts (weights: 8.4 MB fp16 per
  core -- each expert's weights are read exactly once chip-wide).
  Per core: h[f_slice] = gelu(W1[:, f_slice]^T X^T + b1[f_slice]) for
  every token (using that token's expert weights), then the partial
  Y^T = W2[f_slice, :]^T h[f_slice] is DMA'd out in fp16.
  This is perfectly load-balanced regardless of gate skew: no padding,
  every core streams exactly T columns through the PE per layer.
Host: sum the 8 fp16 partials in fp32, add b2, scatter back to [B,L,D].
"""

import sys

if "/opt/trn_rl_repo" not in sys.path:
    sys.path.insert(0, "/opt/trn_rl_repo")

import numpy as np

import concourse.bacc as bacc
import concourse.mybir as mybir
import concourse.tile as tile

D, F, E = 1024, 4096, 4
B, L = 4, 2048
T = B * L
NC = 8
P = 128
KD = D // P          # 8  k-tiles over D
FSL = F // NC        # 512 f-slice per core
NFL = FSL // P       # 4  f-tiles per core
GROUP = 1024         # token-column group (x-load / L2-psum unit)
NG = T // GROUP      # 8 groups

TRACE = False
TRACE_CORES = None
LAST_EXEC_NS = []
LAST_TRACES = []
LAST_RESULTS = []

_cache = {}


def _run(nc, in_maps):
    import os
    import time

    from concourse import bass_utils

    trace = TRACE
    if trace:
        bass_utils.upload_artifacts = lambda d: "local://" + d

    def _go(tr):
        return bass_utils.run_bass_kernel_spmd(
            nc, in_maps, core_ids=list(range(NC)), trace=tr,
            trace_cores=TRACE_CORES,
        )

    def _go_untraced():
        # tracing infra broken (hook import failed): force-disable it for
        # the retry — run_bass_kernel_spmd also ORs in the BASS_TRACE env
        prev = os.environ.get("BASS_NEVER_TRACE")
        os.environ["BASS_NEVER_TRACE"] = "1"
        try:
            return _go(False)
        finally:
            if prev is None:
                os.environ.pop("BASS_NEVER_TRACE", None)
            else:
                os.environ["BASS_NEVER_TRACE"] = prev

    res = None
    for attempt in range(3):
        try:
            res = _go(trace)
            break
        except ModuleNotFoundError:
            trace = False
            res = _go_untraced()
            break
        except Exception as ex:
            # the device occasionally comes up wedged
            # (NRT_EXEC_UNIT_UNRECOVERABLE); retry, resetting the jax
            # backend so the retry gets a fresh PJRT client
            msg = str(ex)
            retriable = "UNRECOVERABLE" in msg or "UNAVAILABLE" in msg
            if attempt == 2 or not retriable:
                raise
            try:
                import jax
                import jax.extend.backend as _jeb

                jax.clear_caches()
                _jeb.clear_backends()
            except Exception:
                pass
            time.sleep(2.0)
    if trace:
        LAST_EXEC_NS.append(res.exec_time_ns)
        LAST_TRACES.append(
            res.instructions_and_trace[1] if res.instructions_and_trace else None
        )
        LAST_RESULTS.append(res)
    return res


def _pieces(counts):
    """Cut the expert-sorted token axis into (expert, col0, len) pieces that
    respect both the 512 grid (PSUM bank) and expert boundaries."""
    bounds = np.cumsum(counts)
    pieces = []
    pos = 0
    for e in range(E):
        end = int(bounds[e])
        while pos < end:
            nxt = min(end, (pos // 512 + 1) * 512)
            pieces.append((e, pos, nxt - pos))
            pos = nxt
    return pieces


def _build_ffn(counts):
    """counts: per-expert token counts (tuple of 4 ints summing to T)."""
    key = ("ffn3", counts)
    if key in _cache:
        return _cache[key]
    f32 = mybir.dt.float32
    f16 = mybir.dt.float16
    pieces = _pieces(counts)
    nc = bacc.Bacc("TRN2", target_bir_lowering=False, debug=False, num_devices=NC)
    # x pre-packed on host into contiguous 512-col half-group blocks:
    # block h holds tokens [h*512, (h+1)*512) as [P, KD, 512] with 8KB
    # contiguous per partition row, so one DMA trigger = 128 descriptor
    # runs (vs 1024 for a strided [D,T] slice, which backed up the DGE
    # ring for ~9us per trigger in v5).
    xt = nc.dram_tensor("xt", (2 * NG, P, KD, 512), f16, kind="ExternalInput")
    # per-core f-slice weights, pre-arranged so the partition dim is first
    w1 = nc.dram_tensor("w1", (P, E, NFL, KD, P), f16, kind="ExternalInput")
    b1 = nc.dram_tensor("b1", (P, E, NFL), f32, kind="ExternalInput")
    w2 = nc.dram_tensor("w2", (P, E, KD, NFL, P), f16, kind="ExternalInput")
    yt = nc.dram_tensor("yt", (D, T), f16, kind="ExternalOutput")

    with tile.TileContext(nc) as tc:
        with (
            tc.tile_pool(name="xs", bufs=3) as xpool,
            tc.tile_pool(name="hs", bufs=1) as hpool,
            tc.tile_pool(name="wp", bufs=1) as wpool,
            tc.tile_pool(name="yp", bufs=6) as ypool,
        ):
            # One PSUM pool for everything: p1_0..3 (L1) + p2_0..3 (L2) = 8
            # banks, so no pool-transition barrier separates the layers.
            # Warm the PE clock (HAM un-throttles after ~3.4us of sustained
            # activity) with dummy matmuls into a p2 bank while the first
            # DMAs are still in flight.
            ps_cm = tc.tile_pool(name="ps", bufs=1, space="PSUM")
            psum = ps_cm.__enter__()
            # warmup operand memset runs on GpSimd (idle at t=0, no slow
            # TENSOR_LOAD init like DVE), so the dummies are gated only on
            # the Tensor engine's own ~1.2us preamble load.
            wsrc = wpool.tile([P, 512], f16, name="wsrc")
            nc.gpsimd.memset(wsrc[:], 0.0)
            wdst = psum.tile([P, 512], f32, name="p2_0")
            # bridge dummies BEFORE the first real chain: the PE queue is
            # in-order, so dummies emitted after the data-gated first matmul
            # could not cover late DMA arrival on hardware. At the cold
            # 1.2GHz clock a 512-col dummy is ~427ns; 11 of them span the
            # ~3.4us HAM ramp and end right as the first 1MB x half-group
            # lands (~12us incl. the ~7us framework preamble).
            for _ in range(11):
                nc.tensor.matmul(
                    wdst[:], wsrc[:, :P], wsrc[:], start=True, stop=True
                )
            # pre-load the Gelu table on the Act engine while DMAs stream
            gwarm = wpool.tile([P, 8], f16, name="gwarm")
            nc.scalar.activation(
                gwarm[:], wsrc[:, :8], mybir.ActivationFunctionType.Gelu
            )

            # weights: per-f-slice triggers for the first expert so the
            # first chains are gated on 256KB, not the full 1MB expert
            e_first = pieces[0][0]
            e_order = [e_first] + [e for e in range(E) if e != e_first]
            w1sb = wpool.tile([P, E, NFL, KD, P], f16, name="w1sb")
            b1sb = wpool.tile([P, E, NFL], f32, name="b1sb")
            w2sb = wpool.tile([P, E, KD, NFL, P], f16, name="w2sb")
            nc.gpsimd.dma_start(w1sb[:, e_first, 0], w1.ap()[:, e_first, 0])
            nc.gpsimd.dma_start(b1sb[:], b1.ap()[:])
            for fl in range(1, NFL):
                nc.gpsimd.dma_start(
                    w1sb[:, e_first, fl], w1.ap()[:, e_first, fl]
                )
            # w1 for the other experts and all of w2 are NOT issued here:
            # they are emitted on the Act queue after group 0 / group 1's
            # gelus (below), so only ~1MB of weights competes with x for
            # HBM during the critical first ~20us.

            # x: one [P, 2, KD, 512] tile per group, loaded as two 512-col
            # half-group descriptors (contiguous 8KB per partition row).
            # Region-level deps let each 512-col piece start as soon as its
            # own half landed.
            xgs = []
            for g in range(NG):
                xg = xpool.tile([P, 2, KD, 512], f16, name="xg")
                for h in range(2):
                    nc.sync.dma_start(xg[:, h], xt.ap()[2 * g + h])
                xgs.append(xg)

            # h[fl] spans all T columns, fp16
            hts = [hpool.tile([P, T], f16, name=f"h{fl}") for fl in range(NFL)]

            # -- layer 1 --
            by_group = [[] for _ in range(NG)]
            for pc in pieces:
                by_group[pc[1] // GROUP].append(pc)
            for g in range(NG):
                xg = xgs[g]
                for (e, c0, ln) in by_group[g]:
                    lo = c0 - g * GROUP
                    hh, off = lo // 512, lo % 512
                    pts = [
                        psum.tile([P, 512], f32, name=f"p1_{fl}")
                        for fl in range(NFL)
                    ]
                    for fl in range(NFL):
                        for k in range(KD):
                            nc.tensor.matmul(
                                pts[fl][:, :ln], w1sb[:, e, fl, k],
                                xg[:, hh, k, off:off + ln],
                                start=(k == 0), stop=(k == KD - 1),
                            )
                        nc.scalar.activation(
                            hts[fl][:, c0:c0 + ln], pts[fl][:, :ln],
                            mybir.ActivationFunctionType.Gelu,
                            bias=b1sb[:, e, fl:fl + 1], scale=1.0,
                        )
                # Deferred weight loads. A plain later dma_start gets
                # HOISTED by the tile scheduler (no deps -> runs at t=0 and
                # steals early HBM bandwidth from x, measured in v5). The
                # 1-col token copy below reads a gelu output of this group
                # and writes into the weight tile, so the full-tile DMA
                # (write-after-write on the token cell, emitted later)
                # cannot be scheduled before this group's compute.
                if g == 0:
                    tok = hts[0][:, 0:1]
                    for e in e_order[1:]:
                        nc.scalar.copy(w1sb[:, e, 0, 0, 0:1], tok)
                        nc.scalar.dma_start(w1sb[:, e], w1.ap()[:, e])
                if g == 1:
                    tok = hts[0][:, GROUP:GROUP + 1]
                    for e in e_order:
                        nc.scalar.copy(w2sb[:, e, 0, 0, 0:1], tok)
                        nc.scalar.dma_start(w2sb[:, e], w2.ap()[:, e])

            # -- layer 2: partial Y^T[d] = sum_fl W2[fl,d]^T H^T[fl] --
            # one chain per (d, piece), rotating over ALL 8 psum banks.
            # Copies alternate Act (ACTIVATE-COPY ~440ns) / DVE (~830ns).
            # Only gpsimd/SP/Act can trigger DMAs; gpsimd's software DGE has
            # ~7us completion latency (it WAS the whole end-of-kernel drain
            # in v3), so it only gets stores in the first 6 of 8 d-tiles.
            l2ps = list(pieces)
            # split the very last piece so the drain ends on small transfers
            (le, lc0, lln) = l2ps[-1]
            if lln > 320:
                h1 = lln - 256
                l2ps[-1:] = [(le, lc0, h1), (le, lc0 + h1, 128),
                             (le, lc0 + h1 + 128, 128)]
            elif lln > 192:
                h1 = lln - 128
                l2ps[-1:] = [(le, lc0, h1), (le, lc0 + h1, lln - h1)]
            banks = ["p2_0", "p2_1", "p2_2", "p2_3",
                     "p1_0", "p1_1", "p1_2", "p1_3"]
            jj = 0
            for d in range(KD):
                plist = l2ps if d == KD - 1 else pieces
                for ip, (e, c0, ln) in enumerate(plist):
                    pt = psum.tile([P, 512], f32, name=banks[jj % 8])
                    for fl in range(NFL):
                        nc.tensor.matmul(
                            pt[:, :ln], w2sb[:, e, d, fl],
                            hts[fl][:, c0:c0 + ln],
                            start=(fl == 0), stop=(fl == NFL - 1),
                        )
                    ys = ypool.tile([P, 512], f16, name="ysb")
                    tail2 = d == KD - 1 and ip >= len(plist) - 3
                    if tail2:
                        # last stores pinned to the HW DGE queues
                        cp = (len(plist) - ip) % 2
                        st = [nc.sync, nc.scalar][(len(plist) - ip) % 2]
                    elif d < KD - 2:
                        cp = jj % 2
                        st = [nc.sync, nc.scalar, nc.gpsimd][jj % 3]
                    else:
                        cp = jj % 2
                        st = [nc.sync, nc.scalar][jj % 2]
                    if cp == 0:
                        nc.scalar.copy(ys[:, :ln], pt[:, :ln])
                    else:
                        nc.vector.tensor_scalar_add(
                            ys[:, :ln], pt[:, :ln], 0.0
                        )
                    st.dma_start(
                        yt.ap()[d * P:(d + 1) * P, c0:c0 + ln],
                        ys[:, :ln],
                    )
                    jj += 1
            ps_cm.__exit__(None, None, None)
    nc.compile()
    _cache[key] = nc
    return nc


def _gate_idx(x, Wg, bg):
    """Replicate the reference gate decision exactly (jax on CPU)."""
    try:
        import jax
        import jax.numpy as jnp

        with jax.default_device(jax.devices("cpu")[0]):
            gl = jnp.einsum(
                "bld,de->ble",
                jnp.asarray(x, dtype=jnp.float32),
                jnp.asarray(Wg, dtype=jnp.float32),
            ) + jnp.asarray(bg, dtype=jnp.float32)
            idx = jnp.argmax(jax.nn.softmax(gl, axis=-1), axis=-1)
        return np.asarray(idx).reshape(T)
    except Exception:
        # fallback: float64 argmax (ties at <1e-7 margin are astronomically
        # unlikely with this data distribution)
        xf = np.asarray(x, dtype=np.float64).reshape(T, D)
        gl = xf @ np.asarray(Wg, dtype=np.float64) + np.asarray(bg, np.float64)
        return np.argmax(gl, axis=1)


def kernel(x, W1, b1, W2, b2, Wg, bg):
    x = np.asarray(x, dtype=np.float32)
    W1 = np.asarray(W1, dtype=np.float32)
    b1 = np.asarray(b1, dtype=np.float32)
    W2 = np.asarray(W2, dtype=np.float32)
    b2 = np.asarray(b2, dtype=np.float32)
    Wg = np.asarray(Wg, dtype=np.float32)
    bg = np.asarray(bg, dtype=np.float32)

    xf = x.reshape(T, D)

    # ---- host routing (exact) ----
    idx = _gate_idx(x, Wg, bg)
    order = np.argsort(idx, kind="stable")
    counts = tuple(int(c) for c in np.bincount(idx, minlength=E))

    # [D, T] sorted, then packed into contiguous half-group blocks
    # [2*NG, P, KD, 512]: block h = tokens [h*512,(h+1)*512), row-major
    # (P, KD, 512) so each partition row is one contiguous 8KB run.
    XTD = xf[order].T.astype(np.float16)                      # [D, T]
    XT16 = np.ascontiguousarray(
        XTD.reshape(KD, P, 2 * NG, 512).transpose(2, 1, 0, 3)
    )                                                         # [16, P, KD, 512]

    # per-core f-slice weight tensors, partition dim first
    # W1: [E, D, F] -> [P(k in tile), E, NC, NFL, KD, P(f)]
    W1t = W1.reshape(E, KD, P, NC, NFL, P).transpose(2, 0, 3, 4, 1, 5)
    W1t = np.ascontiguousarray(W1t).astype(np.float16)  # [P, E, NC, NFL, KD, P]
    b1t = b1.reshape(E, NC, NFL, P).transpose(3, 0, 1, 2)
    b1t = np.ascontiguousarray(b1t)                      # [P, E, NC, NFL]
    # W2: [E, F, D] -> [P(f in tile), E, NC, KD, NFL, P(d)]
    W2t = W2.reshape(E, NC, NFL, P, KD, P).transpose(3, 0, 1, 4, 2, 5)
    W2t = np.ascontiguousarray(W2t).astype(np.float16)  # [P, E, NC, KD, NFL, P]

    in_maps = []
    for c in range(NC):
        in_maps.append({
            "xt": XT16,
            "w1": np.ascontiguousarray(W1t[:, :, c]),
            "b1": np.ascontiguousarray(b1t[:, :, c]),
            "w2": np.ascontiguousarray(W2t[:, :, c]),
        })

    nc2 = _build_ffn(counts)
    res = _run(nc2, in_maps)

    # ---- host reduction: sum partials (fp32), add b2, unsort ----
    acc = res.results[0]["yt"].astype(np.float32)
    for c in range(1, NC):
        acc += res.results[c]["yt"].astype(np.float32)
    ys = acc.T                               # [T, D] in sorted order
    ys += b2[idx[order]]
    out = np.empty((T, D), dtype=np.float32)
    out[order] = ys
    return out.reshape(B, L, D)

